# revision 6
# baseline (speedup 1.0000x reference)
"""Edge-parallel GNN u_mul_v kernel for Trainium2 (8 NeuronCores).

z[e, :] = h[src[e], :] * h[dst[e], :]

Strategy: shard edges across 8 cores (100K each); h (12.8MB) replicated in
HBM as the gather table. The gather primitive is the custom SWDGE
InstDMAGatherAnt (nc.gpsimd.dma_gather): thousands of 256B rows per
instruction, but signed-int16 indices (< 32768). h is therefore addressed as
two tables (h[:32768], h[32768:]) and each core's edges are bucketed on the
host into 4 groups by (src-table, dst-table); the device processes edges in
bucketed order and the host applies the inverse permutation when unsharding.

Perf notes (measured on HW): the 4 SWDGE queues generate/drain descriptors
CONCURRENTLY (~3.6x with 4 queues vs 1), so src/dst gathers round-robin over
all 4 queues with deep buffering. All index tiles are preloaded so no Pool
instruction ever waits on an idx DMA. z is stored fp16 (harness tolerance
2e-2; fp16 product error ~1e-3) halving store traffic; host upcasts.
"""

import numpy as np

N_NODES = 50000
N_EDGES = 800000
D = 64
N_CORES = 8
E_PER_CORE = N_EDGES // N_CORES  # 100000
L = 32768  # int16-addressable rows per gather table
NI = 2048  # edges per tile (per dma_gather call)
G = NI // 128

_cached = {}  # tile structure -> compiled nc


def _build(tiles):
    """tiles: list of (src_hi, dst_hi, ni) per tile (ni % 128 == 0, <= NI)."""
    import concourse.bass as bass
    import concourse.tile as tile
    from concourse import bacc, mybir

    T = len(tiles)
    E_DEV = sum(t[2] for t in tiles)
    nc = bacc.Bacc(
        "TRN2",
        target_bir_lowering=False,
        debug=False,
        num_devices=N_CORES,
        num_swdge_queues=4,
    )
    h_ap = nc.dram_tensor("h", [N_NODES, D], mybir.dt.float32, kind="ExternalInput").ap()
    si_ap = nc.dram_tensor(
        "src_idx", [T, 128, NI // 16], mybir.dt.int16, kind="ExternalInput"
    ).ap()
    di_ap = nc.dram_tensor(
        "dst_idx", [T, 128, NI // 16], mybir.dt.int16, kind="ExternalInput"
    ).ap()
    z_ap = nc.dram_tensor("z", [E_DEV, D], mybir.dt.float16, kind="ExternalOutput").ap()

    tab = {0: h_ap[0:L, :], 1: h_ap[L:N_NODES, :]}

    with tile.TileContext(nc) as tc:
        with (
            tc.tile_pool(name="ix", bufs=1) as ixp,
            tc.tile_pool(name="ga", bufs=4) as gap,
            tc.tile_pool(name="gb", bufs=4) as gbp,
            tc.tile_pool(name="zz", bufs=3) as zp,
        ):
            # Preload every index tile so gathers never wait on idx DMAs.
            sixs, dixs = [], []
            for t in range(T):
                six = ixp.tile([128, NI // 16], mybir.dt.int16, tag=f"six{t}")
                nc.sync.dma_start(six[:], si_ap[t])
                sixs.append(six)
                dix = ixp.tile([128, NI // 16], mybir.dt.int16, tag=f"dix{t}")
                nc.sync.dma_start(dix[:], di_ap[t])
                dixs.append(dix)
            base = 0
            for t, (s_hi, d_hi, ni) in enumerate(tiles):
                g = ni // 128
                ga = gap.tile([128, g, D], mybir.dt.float32, tag="ga")
                nc.gpsimd.dma_gather(
                    out_ap=ga[:],
                    in_ap=tab[s_hi],
                    idxs_ap=sixs[t][:, : ni // 16],
                    num_idxs=ni,
                    num_idxs_reg=ni,
                    elem_size=D,
                    single_packet=False,
                    queue_num=(2 * t) % 4,
                )
                gb = gbp.tile([128, g, D], mybir.dt.float32, tag="gb")
                nc.gpsimd.dma_gather(
                    out_ap=gb[:],
                    in_ap=tab[d_hi],
                    idxs_ap=dixs[t][:, : ni // 16],
                    num_idxs=ni,
                    num_idxs_reg=ni,
                    elem_size=D,
                    single_packet=False,
                    queue_num=(2 * t + 1) % 4,
                )
                zt = zp.tile([128, g, D], mybir.dt.float16, tag="z")
                nc.vector.tensor_mul(zt[:], ga[:], gb[:])
                # device z rows [base : base+ni): slot p*g+gg holds gathered
                # position gg*128+p; contiguous per partition (g*128B runs)
                z_view = z_ap[base : base + ni, :].rearrange(
                    "(p gd) d -> p (gd d)", p=128
                )
                nc.sync.dma_start(z_view, zt[:])
                base += ni
    nc.compile()
    return nc


def _wrap16(a):
    """[ni] int16 gather-sequence -> wrapped [128, ni//16] layout:
    position i lives at partition i%16, slot i//16, replicated x8."""
    w = a.reshape(-1, 16).T
    return np.ascontiguousarray(np.tile(w, (8, 1)))


def _prepare(src, dst):
    """Bucket each core's edges by (src-table, dst-table), sort each bucket by
    src (sequential-ish HBM reads for the src gather), build per-core packed
    int16 index tensors, the shared tile structure (with variable tail tiles),
    and the device-order -> original-edge map."""
    src = np.asarray(src).astype(np.int64)
    dst = np.asarray(dst).astype(np.int64)
    groups = []  # [core][k] -> original edge indices (global), src-sorted
    for c in range(N_CORES):
        lo, hi = c * E_PER_CORE, (c + 1) * E_PER_CORE
        s, d = src[lo:hi], dst[lo:hi]
        g = (s >= L).astype(np.int64) * 2 + (d >= L).astype(np.int64)
        glist = []
        for k in range(4):
            e = np.where(g == k)[0]
            e = e[np.argsort(s[e], kind="stable")]
            glist.append(e + lo)
        groups.append(glist)
    caps = [
        -(-max(len(groups[c][k]) for c in range(N_CORES)) // 128) * 128
        for k in range(4)
    ]
    # Full tiles round-robin across classes (spreads SWDGE queue load);
    # small tail tiles last (shrinks the final drain).
    tiles = []  # (s_hi, d_hi, ni) in device order
    tile_cls = []  # (k, start-within-class)
    cursors = [0, 0, 0, 0]
    emitted = True
    while emitted:
        emitted = False
        for k in range(4):
            if caps[k] - cursors[k] >= NI:
                tiles.append((k >> 1, k & 1, NI))
                tile_cls.append((k, cursors[k]))
                cursors[k] += NI
                emitted = True
    for k in range(4):
        rem = caps[k] - cursors[k]
        if rem > 0:
            tiles.append((k >> 1, k & 1, rem))
            tile_cls.append((k, cursors[k]))
            cursors[k] += rem
    T = len(tiles)
    E_DEV = sum(t[2] for t in tiles)

    tile_bases = np.cumsum([0] + [t[2] for t in tiles])
    class_base = np.cumsum([0] + caps)
    in_maps = []
    dev_orig = np.empty((N_CORES, E_DEV), np.int64)
    for c in range(N_CORES):
        orig = np.full(class_base[-1], -1, np.int64)
        for k in range(4):
            e = groups[c][k]
            orig[class_base[k] : class_base[k] + len(e)] = e
        s_loc = src[np.maximum(orig, 0)]
        d_loc = dst[np.maximum(orig, 0)]
        si = np.zeros((T, 128, NI // 16), np.int16)
        di = np.zeros((T, 128, NI // 16), np.int16)
        for t, ((s_hi, d_hi, ni), (k, start)) in enumerate(zip(tiles, tile_cls)):
            b = class_base[k] + start
            zb = tile_bases[t]
            s16 = np.where(
                orig[b : b + ni] >= 0, s_loc[b : b + ni] - s_hi * L, 0
            ).astype(np.int16)
            d16 = np.where(
                orig[b : b + ni] >= 0, d_loc[b : b + ni] - d_hi * L, 0
            ).astype(np.int16)
            si[t, :, : ni // 16] = _wrap16(s16)
            di[t, :, : ni // 16] = _wrap16(d16)
            # device slot p*(ni//128)+g holds gathered position g*128+p
            tmap = np.arange(ni).reshape(ni // 128, 128).T.reshape(-1)
            dev_orig[c, zb : zb + ni] = orig[b : b + ni][tmap]
        in_maps.append({"si": si, "di": di})
    return tiles, in_maps, dev_orig


def _get_nc(tiles):
    key = tuple(tiles)
    if key not in _cached:
        _cached[key] = _build(list(key))
    return _cached[key]


def _make_in_maps(h, src, dst):
    tiles, idx_maps, dev_orig = _prepare(src, dst)
    h32 = np.ascontiguousarray(h, dtype=np.float32)
    in_maps = [
        {"h": h32, "src_idx": m["si"], "dst_idx": m["di"]} for m in idx_maps
    ]
    return tiles, in_maps, dev_orig


def kernel(h, src, dst):
    from concourse import bass_utils

    tiles, in_maps, dev_orig = _make_in_maps(h, src, dst)
    nc = _get_nc(tiles)
    res = bass_utils.run_bass_kernel_spmd(nc, in_maps, list(range(N_CORES)))
    out = np.empty((N_EDGES, D), np.float32)
    for c in range(N_CORES):
        zc = res.results[c]["z"]
        valid = dev_orig[c] >= 0
        out[dev_orig[c][valid]] = zc[valid].astype(np.float32)
    return out


# revision 10
# speedup vs baseline: 1.1146x; 1.1146x over previous
"""Edge-parallel GNN u_mul_v kernel for Trainium2 (8 NeuronCores).

z[e, :] = h[src[e], :] * h[dst[e], :]

Shard edges across 8 cores (100K each); h (12.8MB) replicated in HBM as the
gather table, addressed as two int16-indexable tables (h[:32768], h[32768:]).
Rows are fetched with the SWDGE InstDMAGatherAnt (nc.gpsimd.dma_gather);
measured desc-generation runs ~2.2ns/descriptor aggregate (4 queues), so
descriptor COUNT is the wall.

Src-side token pairing cuts descriptors ~19%: edges are sorted by src, and
two edges whose src rows fall in the same 512B pair-token (rows 2t, 2t+1)
share ONE elem=128 descriptor. The pair's members land in z slots (p, 2m)
and (p, 2m+1) -- same partition, adjacent slots -- so the multiply reads the
shared gather slot through static strided views with per-tile parity offsets
(pairs bucketed by (parity1<=parity2) pattern: 3 patterns x 4 table-classes).
Unpairable edges go through the original per-edge path (elem=64).

z is stored fp16 (gate 2e-2; fp16 product error ~1e-3); host upcasts.
All index tiles are preloaded; gathers round-robin the 4 SWDGE queues.
"""

import numpy as np

N_NODES = 50000
N_EDGES = 800000
D = 64
N_CORES = 8
E_PER_CORE = N_EDGES // N_CORES  # 100000
L = 32768  # int16-addressable rows per gather table
NP = 2048  # pairs per pair-tile (src gather descs; dst gather = 2*NP)
NS = 4096  # edges per single-tile

_cached = {}


def _build(tiles):
    """tiles: list of ("p", s_hi, d_hi, offA, offB, npairs) or
    ("s", s_hi, d_hi, ni)."""
    import concourse.bass as bass
    import concourse.tile as tile
    from concourse import bacc, mybir

    T = len(tiles)
    E_DEV = sum((2 * t[5]) if t[0] == "p" else t[3] for t in tiles)
    nc = bacc.Bacc(
        "TRN2",
        target_bir_lowering=False,
        debug=False,
        num_devices=N_CORES,
        num_swdge_queues=4,
    )
    h_ap = nc.dram_tensor("h", [N_NODES, D], mybir.dt.float32, kind="ExternalInput").ap()
    # per-tile idx tensors are packed into one [T, 128, NS//16] input;
    # pair tiles use [:, :NP//16] for src and [:, :2*NP//16] for dst.
    si_ap = nc.dram_tensor(
        "src_idx", [T, 128, NS // 16], mybir.dt.int16, kind="ExternalInput"
    ).ap()
    di_ap = nc.dram_tensor(
        "dst_idx", [T, 128, NS // 16], mybir.dt.int16, kind="ExternalInput"
    ).ap()
    z_ap = nc.dram_tensor("z", [E_DEV, D], mybir.dt.float16, kind="ExternalOutput").ap()

    tab = {0: h_ap[0:L, :], 1: h_ap[L:N_NODES, :]}
    ptab = {
        0: h_ap[0:L, :].rearrange("(n two) d -> n (two d)", two=2),
        1: h_ap[L:N_NODES, :].rearrange("(n two) d -> n (two d)", two=2),
    }

    with tile.TileContext(nc) as tc:
        with (
            tc.tile_pool(name="ix", bufs=1) as ixp,
            tc.tile_pool(name="ga", bufs=4) as gap,
            tc.tile_pool(name="gb", bufs=4) as gbp,
            tc.tile_pool(name="zz", bufs=3) as zp,
        ):
            sixs, dixs = [], []
            for t, tl in enumerate(tiles):
                ns_ = (tl[5] // 16) if tl[0] == "p" else (tl[3] // 16)
                nd_ = (2 * tl[5] // 16) if tl[0] == "p" else (tl[3] // 16)
                six = ixp.tile([128, ns_], mybir.dt.int16, tag=f"six{t}")
                nc.sync.dma_start(six[:], si_ap[t][:, :ns_])
                sixs.append(six)
                dix = ixp.tile([128, nd_], mybir.dt.int16, tag=f"dix{t}")
                nc.sync.dma_start(dix[:], di_ap[t][:, :nd_])
                dixs.append(dix)
            base = 0
            for t, tl in enumerate(tiles):
                if tl[0] == "p":
                    _, s_hi, d_hi, offA, offB, npr = tl
                    m = npr // 128
                    ga = gap.tile([128, m, 2 * D], mybir.dt.float32, tag="gap")
                    nc.gpsimd.dma_gather(
                        out_ap=ga[:], in_ap=ptab[s_hi], idxs_ap=sixs[t][:],
                        num_idxs=npr, num_idxs_reg=npr, elem_size=2 * D,
                        single_packet=False, queue_num=t % 4,
                    )
                    gb = gbp.tile([128, 2 * m, D], mybir.dt.float32, tag="gbp")
                    nc.gpsimd.dma_gather(
                        out_ap=gb[:], in_ap=tab[d_hi], idxs_ap=dixs[t][:],
                        num_idxs=2 * npr, num_idxs_reg=2 * npr, elem_size=D,
                        single_packet=False, queue_num=(t + 2) % 4,
                    )
                    zt = zp.tile([128, 2 * m, D], mybir.dt.float16, tag="zp")
                    gbv = gb[:].rearrange("p (m two) d -> p m two d", two=2)
                    ztv = zt[:].rearrange("p (m two) d -> p m two d", two=2)
                    nc.vector.tensor_mul(
                        ztv[:, :, 0, :], ga[:, :, offA * D : (offA + 1) * D],
                        gbv[:, :, 0, :],
                    )
                    nc.vector.tensor_mul(
                        ztv[:, :, 1, :], ga[:, :, offB * D : (offB + 1) * D],
                        gbv[:, :, 1, :],
                    )
                    ni = 2 * npr
                else:
                    _, s_hi, d_hi, ni = tl
                    g = ni // 128
                    ga = gap.tile([128, g, D], mybir.dt.float32, tag="gas")
                    nc.gpsimd.dma_gather(
                        out_ap=ga[:], in_ap=tab[s_hi], idxs_ap=sixs[t][:],
                        num_idxs=ni, num_idxs_reg=ni, elem_size=D,
                        single_packet=False, queue_num=t % 4,
                    )
                    gb = gbp.tile([128, g, D], mybir.dt.float32, tag="gbs")
                    nc.gpsimd.dma_gather(
                        out_ap=gb[:], in_ap=tab[d_hi], idxs_ap=dixs[t][:],
                        num_idxs=ni, num_idxs_reg=ni, elem_size=D,
                        single_packet=False, queue_num=(t + 2) % 4,
                    )
                    zt = zp.tile([128, g, D], mybir.dt.float16, tag="zs")
                    nc.vector.tensor_mul(zt[:], ga[:], gb[:])
                z_view = z_ap[base : base + ni, :].rearrange(
                    "(p gd) d -> p (gd d)", p=128
                )
                nc.sync.dma_start(z_view, zt[:])
                base += ni
    nc.compile()
    return nc


def _wrap16(a):
    w = np.asarray(a, np.int16).reshape(-1, 16).T
    return np.ascontiguousarray(np.tile(w, (8, 1)))


def _pair_class(s_loc, eids):
    """Edges of one (core, table-class), s_loc sorted ascending.
    Returns pairs[npair, 2] (eid), pair_tok[npair], pat[npair] in {0,1,2},
    singles eid array, singles s_loc array."""
    tok = s_loc >> 1
    par = (s_loc & 1).astype(np.int64)
    n = len(tok)
    if n == 0:
        z = np.zeros(0, np.int64)
        return z.reshape(0, 2), z, z, z, z
    newrun = np.r_[True, tok[1:] != tok[:-1]]
    starts = np.where(newrun)[0]
    lens = np.diff(np.r_[starts, n])
    run_start = np.repeat(starts, lens)
    run_len = np.repeat(lens, lens)
    pos = np.arange(n) - run_start
    is_first = (pos % 2 == 0) & (pos + 1 < run_len)
    is_single = (pos % 2 == 0) & (pos + 1 >= run_len)
    a = np.where(is_first)[0]
    b = a + 1
    pairs = np.stack([eids[a], eids[b]], axis=1)
    # sorted src => par[a] <= par[b]; pattern: 00->0, 01->1, 11->2
    pat = par[a] + par[b]
    sing = np.where(is_single)[0]
    return pairs, tok[a], pat, eids[sing], s_loc[sing]


def _prepare(src, dst):
    src = np.asarray(src).astype(np.int64)
    dst = np.asarray(dst).astype(np.int64)
    # bucket types: 12 pair buckets (class k in 0..3, pattern in 0..2) and
    # 4 single buckets (class k). Collect per-core contents.
    pair_b = [[None] * 12 for _ in range(N_CORES)]  # (pairs[n,2], tok[n])
    sing_b = [[None] * 4 for _ in range(N_CORES)]  # eids[n]
    for c in range(N_CORES):
        lo, hi = c * E_PER_CORE, (c + 1) * E_PER_CORE
        s, d = src[lo:hi], dst[lo:hi]
        cls = (s >= L).astype(np.int64) * 2 + (d >= L).astype(np.int64)
        for k in range(4):
            e = np.where(cls == k)[0]
            s_hi = k >> 1
            sl = s[e] - s_hi * L
            o = np.argsort(sl, kind="stable")
            e, sl = e[o] + lo, sl[o]
            pairs, ptok, pat, se, _ss = _pair_class(sl, e)
            for p in range(3):
                m = pat == p
                pair_b[c][k * 3 + p] = (pairs[m], ptok[m])
            sing_b[c][k] = se
    pcaps = [
        -(-max(len(pair_b[c][j][0]) for c in range(N_CORES)) // 128) * 128
        for j in range(12)
    ]
    scaps = [
        -(-max(len(sing_b[c][k]) for c in range(N_CORES)) // 128) * 128
        for k in range(4)
    ]
    # tiles: full pair tiles and full single tiles round-robin, tails last.
    PAT_OFF = {0: (0, 0), 1: (0, 1), 2: (1, 1)}
    tiles = []
    meta = []  # ("p", j, start) / ("s", k, start)
    pcur = [0] * 12
    scur = [0] * 4
    emitted = True
    while emitted:
        emitted = False
        for j in range(12):
            if pcaps[j] - pcur[j] >= NP:
                k, p = j // 3, j % 3
                tiles.append(("p", k >> 1, k & 1, *PAT_OFF[p], NP))
                meta.append(("p", j, pcur[j]))
                pcur[j] += NP
                emitted = True
        for k in range(4):
            if scaps[k] - scur[k] >= NS:
                tiles.append(("s", k >> 1, k & 1, NS))
                meta.append(("s", k, scur[k]))
                scur[k] += NS
                emitted = True
    for j in range(12):
        rem = pcaps[j] - pcur[j]
        if rem > 0:
            k, p = j // 3, j % 3
            tiles.append(("p", k >> 1, k & 1, *PAT_OFF[p], rem))
            meta.append(("p", j, pcur[j]))
            pcur[j] += rem
    for k in range(4):
        rem = scaps[k] - scur[k]
        if rem > 0:
            tiles.append(("s", k >> 1, k & 1, rem))
            meta.append(("s", k, scur[k]))
            scur[k] += rem
    T = len(tiles)
    tile_ni = [(2 * t[5]) if t[0] == "p" else t[3] for t in tiles]
    E_DEV = sum(tile_ni)
    tile_bases = np.cumsum([0] + tile_ni)

    in_maps = []
    dev_orig = np.empty((N_CORES, E_DEV), np.int64)
    for c in range(N_CORES):
        si = np.zeros((T, 128, NS // 16), np.int16)
        di = np.zeros((T, 128, NS // 16), np.int16)
        for t, (tl, (kind, jk, start)) in enumerate(zip(tiles, meta)):
            zb = tile_bases[t]
            if kind == "p":
                npr = tl[5]
                pairs_all, tok_all = pair_b[c][jk]
                seg_p = np.full((npr, 2), -1, np.int64)
                seg_t = np.zeros(npr, np.int64)
                avail = max(0, min(len(tok_all) - start, npr))
                if avail > 0:
                    seg_p[:avail] = pairs_all[start : start + avail]
                    seg_t[:avail] = tok_all[start : start + avail]
                si[t, :, : npr // 16] = _wrap16(seg_t)
                # dst positions: t2 -> (p=t2%128, slot=t2//128); edge at
                # (p, 2q+w) = pair[q*128+p][w]
                m = npr // 128
                ebpj = np.empty((128, 2 * m), np.int64)
                pr = seg_p.reshape(m, 128, 2)
                for w in range(2):
                    ebpj[:, w::2] = pr[:, :, w].T
                dseq = ebpj.T.reshape(-1)  # position order
                d_hi = tl[2]
                dloc = np.where(dseq >= 0, dst[np.maximum(dseq, 0)] - d_hi * L, 0)
                di[t, :, : 2 * npr // 16] = _wrap16(dloc)
                # device row r = p*(2m) + j ; edge = pair[(j//2)*128+p][j%2]
                rr = np.arange(2 * npr)
                p_, j_ = rr // (2 * m), rr % (2 * m)
                dev_orig[c, zb : zb + 2 * npr] = seg_p[(j_ // 2) * 128 + p_, j_ % 2]
            else:
                ni = tl[3]
                se = sing_b[c][jk]
                seg = np.full(ni, -1, np.int64)
                avail = max(0, min(len(se) - start, ni))
                if avail > 0:
                    seg[:avail] = se[start : start + avail]
                s_hi, d_hi = tl[1], tl[2]
                sloc = np.where(seg >= 0, src[np.maximum(seg, 0)] - s_hi * L, 0)
                dloc = np.where(seg >= 0, dst[np.maximum(seg, 0)] - d_hi * L, 0)
                si[t, :, : ni // 16] = _wrap16(sloc)
                di[t, :, : ni // 16] = _wrap16(dloc)
                g = ni // 128
                tmap = np.arange(ni).reshape(g, 128).T.reshape(-1)
                dev_orig[c, zb : zb + ni] = seg[tmap]
        in_maps.append({"si": si, "di": di})
    return tiles, in_maps, dev_orig


def _get_nc(tiles):
    key = tuple(tiles)
    if key not in _cached:
        _cached[key] = _build(list(key))
    return _cached[key]


def _make_in_maps(h, src, dst):
    tiles, idx_maps, dev_orig = _prepare(src, dst)
    h32 = np.ascontiguousarray(h, dtype=np.float32)
    in_maps = [
        {"h": h32, "src_idx": m["si"], "dst_idx": m["di"]} for m in idx_maps
    ]
    return tiles, in_maps, dev_orig


def kernel(h, src, dst):
    from concourse import bass_utils

    tiles, in_maps, dev_orig = _make_in_maps(h, src, dst)
    nc = _get_nc(tiles)
    res = bass_utils.run_bass_kernel_spmd(nc, in_maps, list(range(N_CORES)))
    out = np.empty((N_EDGES, D), np.float32)
    for c in range(N_CORES):
        zc = res.results[c]["z"]
        valid = dev_orig[c] >= 0
        out[dev_orig[c][valid]] = zc[valid].astype(np.float32)
    return out
